# revision 2
# baseline (speedup 1.0000x reference)
"""Conv2d 3x3 (B=32, Cin=128, H=W=56, Cout=256, pad=1, stride=1) + bias.

1D Winograd F(2,3) along the x-axis, data-parallel over batch across 8 cores
(4 images/core). Per output row, each pair of output columns (2j, 2j+1) is
computed from 4 transformed input planes instead of 6 raw-tap streams:

  z = x[2j-1 .. 2j+2]            (band cols 2j .. 2j+3, col0/col57 zero pad)
  d0 = z0-z2, d1 = z1+z2, d2 = z2-z1, d3 = z1-z3      (input transform, DVE)
  g0 = w0, g1 = (w0+w1+w2)/2, g2 = (w0-w1+w2)/2, g3 = w2   (host)
  m_p[co] = sum_ky sum_ci g_p[ky][ci,co] d_p[ci, r+ky]     (PE, PSUM fp32)
  y0 = m0+m1+m2+bias,  y1 = m1-m2-m3+bias                  (ACT+DVE+GpSimd)

This cuts PE streamed columns per output from 9 to 6 (direct conv streams
each tap separately; winograd shares the m-planes between the two outputs).
The output transform is split across three otherwise-idle engines with at
most one PSUM operand per op (PSUM sources cap DVE at 1x):
  ACT:    t0 = fp16(m0), t = fp32(m1 + bias), t3 = fp16(m3)
  DVE:    u = fp16(m2 + t), [cb0] y0 = u + t0   (+ all 4 input-transform ops)
  GpSimd: v = fp16(t - m2), y1 = v - t3, [cb1] y0 = u + t0

Row tiles of 16 (56 = 16*3 + 8); x staged as overlapping 18-row bands.
Vertical padding handled by clipping matmul rows at the image edges
(PSUM has_written first-touch-overwrite semantics), horizontal padding by
zeroed band columns 0 and 57.
"""

import numpy as np

import concourse.bass as bass
import concourse.mybir as mybir
import concourse.tile as tile
from concourse import bacc
from concourse.bass_utils import run_bass_kernel_spmd

B, C_IN, H, W = 32, 128, 56, 56
C_OUT, KSZ = 256, 3
N_CORES = 8
B_LOC = B // N_CORES  # 4 images per core
RTS = [16, 16, 16, 8]  # output rows per tile (56 = 3*16 + 8)
NT = len(RTS)
CBLKS = C_OUT // 128  # 2
NPOS = 4  # winograd F(2,3) positions
TC = W // 2  # 28 output-column pairs
BC = 58  # band columns (56 + 2 pad)

MM_DT = mybir.dt.float16
WARM_COUNT = 40


def build_nc():
    nc = bacc.Bacc(None, target_bir_lowering=False)
    # x host-padded to 58 cols (zero col 0 and 57) so bands DMA whole rows
    x = nc.dram_tensor("x", [B_LOC, C_IN, H, BC], MM_DT, kind="ExternalInput")
    g = nc.dram_tensor("g", [C_IN, CBLKS, KSZ, NPOS, 128], MM_DT, kind="ExternalInput")
    bias = nc.dram_tensor("bias", [128, CBLKS], mybir.dt.float32, kind="ExternalInput")
    # y0 (even cols) and y1 (odd cols) planes; host interleaves them.
    out = nc.dram_tensor(
        "out", [B_LOC, CBLKS, 2, 128, H, TC], mybir.dt.float16, kind="ExternalOutput"
    )

    bands = [(b, t) for b in range(B_LOC) for t in range(NT)]  # 16 bands

    def band_rows(t):
        r0 = max(0, 16 * t - 1)
        r1 = min(H, 16 * t + RTS[t] + 1)
        l0 = 1 if t == 0 else 0
        return r0, r1, l0

    with tile.TileContext(nc) as tc:
        with (
            tc.tile_pool(name="xin", bufs=5) as xpool,
            tc.tile_pool(name="dpool", bufs=6) as dpool,
            tc.tile_pool(name="wpool", bufs=1) as wpool,
            tc.tile_pool(name="aux", bufs=6) as auxpool,
            tc.tile_pool(name="psum", bufs=2, space="PSUM") as psum_pool,
            tc.tile_pool(name="outp", bufs=8) as opool,
        ):
            xts = {}
            dts = {}

            def stage_band(bi):
                """DMA host-padded x rows for band bi."""
                b, t = bands[bi]
                xt = xpool.tile([C_IN, 18, BC], MM_DT)
                r0, r1, l0 = band_rows(t)
                nc.sync.dma_start(xt[:, l0 : l0 + (r1 - r0), :], x[b, :, r0:r1, :])
                xts[bi] = xt

            def dtrans(bi):
                """Input transform band bi -> 4 d-plane tiles (all on DVE)."""
                _, t = bands[bi]
                nrb = RTS[t] + 2
                xt = xts[bi]
                ds = [
                    dpool.tile([C_IN, 18, TC], MM_DT, name=f"d{p}")
                    for p in range(NPOS)
                ]
                z0 = xt[:, :nrb, 0:55:2]
                z1 = xt[:, :nrb, 1:56:2]
                z2 = xt[:, :nrb, 2:57:2]
                z3 = xt[:, :nrb, 3:58:2]
                # d0 on DVE; d1-d3 on GpSimd (SBUF-only engine, DVE is the
                # only vector engine allowed to touch PSUM so keep its slack)
                nc.vector.tensor_sub(ds[0][:, :nrb, :], z0, z2)
                nc.gpsimd.tensor_add(ds[1][:, :nrb, :], z1, z2)
                nc.gpsimd.tensor_sub(ds[2][:, :nrb, :], z2, z1)
                nc.gpsimd.tensor_sub(ds[3][:, :nrb, :], z1, z3)
                dts[bi] = ds

            # --- startup: band0 first, then weights, band1; bias on scalar ring.
            g_sb = wpool.tile([C_IN, CBLKS, KSZ, NPOS, 128], MM_DT)
            bias_sb = wpool.tile([128, CBLKS], mybir.dt.float32)
            stage_band(0)
            nc.sync.dma_start(g_sb[:], g[:])
            stage_band(1)
            nc.scalar.dma_start(bias_sb[:], bias[:, :])

            # HAM pre-warm (see baseline): dummy matmuls on a DVE-zeroed tile.
            warm = wpool.tile([C_IN, 64], MM_DT)
            # share the m0 psum ring so warm fits in the 8 banks
            warm_ps = psum_pool.tile([128, 16, TC], mybir.dt.float32, name="m0")
            nc.vector.memset(warm[:].bitcast(mybir.dt.uint16), 0)
            for _ in range(WARM_COUNT):
                nc.tensor.matmul(
                    warm_ps[:64, 0:2, :], warm[:, :64], warm[:, :56],
                    start=True, stop=True, skip_group_check=True,
                )

            dtrans(0)

            for bi, (b, t) in enumerate(bands):
                if bi + 2 < len(bands):
                    stage_band(bi + 2)
                if bi + 1 < len(bands):
                    dtrans(bi + 1)
                nr = RTS[t]
                ds = dts[bi]
                for cb in range(CBLKS):
                    ms = [
                        psum_pool.tile(
                            [128, 16, TC], mybir.dt.float32, name=f"m{p}"
                        )
                        for p in range(NPOS)
                    ]
                    for pos in range(NPOS):
                        for ky in range(KSZ):
                            a2 = 1 if (t == 0 and ky == 0) else 0
                            b2 = nr - 1 if (t == NT - 1 and ky == 2) else nr
                            nc.tensor.matmul(
                                ms[pos][:, a2:b2, :],
                                g_sb[:, cb, ky, pos, :],
                                ds[pos][:, a2 + ky : b2 + ky, :],
                                start=(ky == 0),
                                stop=(ky == KSZ - 1),
                                skip_group_check=True,
                            )
                    # output transform: y0 = m0+m1+m2+b, y1 = m1-m2-m3+b.
                    # ACT extracts m0/m1+bias/m3 to fp16 SBUF (releases PSUM
                    # early); DVE copies m2 (PSUM copy = 2x, cheaper than
                    # tensor_tensor) then combines at the fp16 SBUF 2x rate.
                    t0 = auxpool.tile([128, 16, TC], mybir.dt.float16)
                    tt = auxpool.tile([128, 16, TC], mybir.dt.float32)
                    t3 = auxpool.tile([128, 16, TC], mybir.dt.float16)
                    uu = auxpool.tile([128, 16, TC], mybir.dt.float16)
                    vv = auxpool.tile([128, 16, TC], mybir.dt.float16)
                    y0 = opool.tile([128, 16, TC], mybir.dt.float16)
                    y1 = opool.tile([128, 16, TC], mybir.dt.float16)
                    idn = mybir.ActivationFunctionType.Identity
                    nc.scalar.activation(t0[:, :nr], ms[0][:, :nr], idn, bias=0.0, scale=1.0)
                    nc.scalar.activation(
                        tt[:, :nr], ms[1][:, :nr], idn, bias=bias_sb[:, cb : cb + 1], scale=1.0
                    )
                    nc.scalar.activation(t3[:, :nr], ms[3][:, :nr], idn, bias=0.0, scale=1.0)
                    nc.vector.tensor_add(uu[:, :nr], ms[2][:, :nr], tt[:, :nr])
                    nc.vector.tensor_sub(vv[:, :nr], tt[:, :nr], ms[2][:, :nr])
                    nc.vector.tensor_add(y0[:, :nr], uu[:, :nr], t0[:, :nr])
                    nc.vector.tensor_sub(y1[:, :nr], vv[:, :nr], t3[:, :nr])
                    r0o = 16 * t
                    nc.sync.dma_start(out[b, cb, 0, :, r0o : r0o + nr, :], y0[:, :nr])
                    nc.sync.dma_start(out[b, cb, 1, :, r0o : r0o + nr, :], y1[:, :nr])
    nc.finalize()
    return nc


def prep_inputs(x, weight, bias):
    # weight (256,128,3,3) -> winograd F(2,3) x-transform, [ci, cb, ky, pos, co_l]
    w6 = weight.reshape(CBLKS, 128, C_IN, KSZ, KSZ).astype(np.float32)
    g0 = w6[..., 0]
    g1 = (w6[..., 0] + w6[..., 1] + w6[..., 2]) * 0.5
    g2 = (w6[..., 0] - w6[..., 1] + w6[..., 2]) * 0.5
    g3 = w6[..., 2]
    G = np.stack([g0, g1, g2, g3], axis=-1)  # [cb, co, ci, ky, pos]
    g_host = np.ascontiguousarray(G.transpose(2, 0, 3, 4, 1), dtype=np.float16)
    bias_r = np.ascontiguousarray(bias.reshape(CBLKS, 128).T, dtype=np.float32)
    xpad = np.zeros((B, C_IN, H, BC), dtype=np.float16)
    xpad[:, :, :, 1:57] = x
    in_maps = []
    for c in range(N_CORES):
        in_maps.append(
            {
                "x": np.ascontiguousarray(xpad[c * B_LOC : (c + 1) * B_LOC]),
                "g": g_host,
                "bias": bias_r,
            }
        )
    return in_maps


_NC_CACHE = {}


def run(x, weight, bias, trace=False, nc=None, tmpdir=None):
    if nc is None:
        nc = _NC_CACHE.get("nc")
        if nc is None:
            nc = _NC_CACHE["nc"] = build_nc()
    in_maps = prep_inputs(np.asarray(x), np.asarray(weight), np.asarray(bias))
    res = run_bass_kernel_spmd(
        nc, in_maps, core_ids=list(range(N_CORES)), trace=trace, tmpdir=tmpdir
    )
    # device out: [B_LOC, CBLKS, 2, 128, 56, 28] fp16 (q = col parity)
    # -> [B, C_OUT, H, W] fp32 on host
    outs = []
    for r in res.results:
        a = np.asarray(r["out"])  # [B_LOC, 2, 2, 128, 56, 28]
        a = a.transpose(0, 1, 3, 4, 5, 2)  # [B_LOC, cb, 128, 56, 28, 2]
        outs.append(a.reshape(B_LOC, C_OUT, H, W))
    out = np.concatenate(outs, axis=0).astype(np.float32)
    return out, res


def kernel(x, weight, bias):
    out, _ = run(x, weight, bias, trace=False)
    return out


if __name__ == "__main__":
    rng = np.random.default_rng(0)
    x = rng.standard_normal((B, C_IN, H, W), dtype=np.float32)
    w = (rng.standard_normal((C_OUT, C_IN, KSZ, KSZ), dtype=np.float32) * 0.05).astype(
        np.float32
    )
    b = rng.standard_normal((C_OUT,), dtype=np.float32)
    out = kernel(x, w, b)
    print(out.shape, out.dtype)


# revision 3
# speedup vs baseline: 1.0344x; 1.0344x over previous
"""Conv2d 3x3 (B=32, Cin=128, H=W=56, Cout=256, pad=1, stride=1) + bias.

1D Winograd F(2,3) along the x-axis, data-parallel over batch across 8 cores
(4 images/core). Per output row, each pair of output columns (2j, 2j+1) is
computed from 4 transformed input planes instead of 6 raw-tap streams:

  z = x[2j-1 .. 2j+2]            (band cols 2j .. 2j+3, col0/col57 zero pad)
  d0 = z0-z2, d1 = z1+z2, d2 = z2-z1, d3 = z1-z3      (input transform, DVE)
  g0 = w0, g1 = (w0+w1+w2)/2, g2 = (w0-w1+w2)/2, g3 = w2   (host)
  m_p[co] = sum_ky sum_ci g_p[ky][ci,co] d_p[ci, r+ky]     (PE, PSUM fp32)
  y0 = m0+m1+m2+bias,  y1 = m1-m2-m3+bias                  (ACT+DVE+GpSimd)

This cuts PE streamed columns per output from 9 to 6 (direct conv streams
each tap separately; winograd shares the m-planes between the two outputs).
The output transform is split across three otherwise-idle engines with at
most one PSUM operand per op (PSUM sources cap DVE at 1x):
  ACT:    t0 = fp16(m0), t = fp32(m1 + bias), t3 = fp16(m3)
  DVE:    u = fp16(m2 + t), [cb0] y0 = u + t0   (+ all 4 input-transform ops)
  GpSimd: v = fp16(t - m2), y1 = v - t3, [cb1] y0 = u + t0

Row tiles of 16 (56 = 16*3 + 8); x staged as overlapping 18-row bands.
Vertical padding handled by clipping matmul rows at the image edges
(PSUM has_written first-touch-overwrite semantics), horizontal padding by
zeroed band columns 0 and 57.
"""

import numpy as np

import concourse.bass as bass
import concourse.mybir as mybir
import concourse.tile as tile
from concourse import bacc
from concourse.bass_utils import run_bass_kernel_spmd

B, C_IN, H, W = 32, 128, 56, 56
C_OUT, KSZ = 256, 3
N_CORES = 8
B_LOC = B // N_CORES  # 4 images per core
RTS = [16, 16, 16, 8]  # output rows per tile (56 = 3*16 + 8)
NT = len(RTS)
CBLKS = C_OUT // 128  # 2
NPOS = 4  # winograd F(2,3) positions
TC = W // 2  # 28 output-column pairs
BC = 58  # band columns (56 + 2 pad)

MM_DT = mybir.dt.float16
WARM_COUNT = 55


def build_nc():
    nc = bacc.Bacc(None, target_bir_lowering=False)
    # x host-padded to 58 cols (zero col 0 and 57) so bands DMA whole rows
    x = nc.dram_tensor("x", [B_LOC, C_IN, H, BC], MM_DT, kind="ExternalInput")
    g = nc.dram_tensor("g", [C_IN, CBLKS, KSZ, NPOS, 128], MM_DT, kind="ExternalInput")
    bias = nc.dram_tensor("bias", [128, CBLKS], mybir.dt.float32, kind="ExternalInput")
    # y0 (even cols) and y1 (odd cols) planes; host interleaves them.
    out = nc.dram_tensor(
        "out", [B_LOC, CBLKS, 2, 128, H, TC], mybir.dt.float16, kind="ExternalOutput"
    )

    bands = [(b, t) for b in range(B_LOC) for t in range(NT)]  # 16 bands

    def band_rows(t):
        r0 = max(0, 16 * t - 1)
        r1 = min(H, 16 * t + RTS[t] + 1)
        l0 = 1 if t == 0 else 0
        return r0, r1, l0

    with tile.TileContext(nc) as tc:
        with (
            tc.tile_pool(name="xin", bufs=5) as xpool,
            tc.tile_pool(name="dpool", bufs=6) as dpool,
            tc.tile_pool(name="wpool", bufs=1) as wpool,
            tc.tile_pool(name="aux", bufs=6) as auxpool,
            tc.tile_pool(name="psum", bufs=2, space="PSUM") as psum_pool,
            tc.tile_pool(name="outp", bufs=8) as opool,
        ):
            xts = {}
            dts = {}

            def stage_band(bi):
                """DMA host-padded x rows for band bi."""
                b, t = bands[bi]
                xt = xpool.tile([C_IN, 18, BC], MM_DT)
                r0, r1, l0 = band_rows(t)
                nc.sync.dma_start(xt[:, l0 : l0 + (r1 - r0), :], x[b, :, r0:r1, :])
                xts[bi] = xt

            def dtrans(bi):
                """Input transform band bi -> 4 d-plane tiles (all on DVE)."""
                _, t = bands[bi]
                nrb = RTS[t] + 2
                xt = xts[bi]
                ds = [
                    dpool.tile([C_IN, 18, TC], MM_DT, name=f"d{p}")
                    for p in range(NPOS)
                ]
                z0 = xt[:, :nrb, 0:55:2]
                z1 = xt[:, :nrb, 1:56:2]
                z2 = xt[:, :nrb, 2:57:2]
                z3 = xt[:, :nrb, 3:58:2]
                # d0 on DVE; d1-d3 on GpSimd (SBUF-only engine, DVE is the
                # only vector engine allowed to touch PSUM so keep its slack).
                # Band 0 is latency-critical: alternate engines so the first
                # group's pos-planes land in matmul order instead of
                # serializing ~3.3us on GpSimd.
                if bi == 0:
                    nc.vector.tensor_sub(ds[0][:, :nrb, :], z0, z2)
                    nc.gpsimd.tensor_add(ds[1][:, :nrb, :], z1, z2)
                    nc.vector.tensor_sub(ds[2][:, :nrb, :], z2, z1)
                    nc.gpsimd.tensor_sub(ds[3][:, :nrb, :], z1, z3)
                else:
                    nc.vector.tensor_sub(ds[0][:, :nrb, :], z0, z2)
                    nc.gpsimd.tensor_add(ds[1][:, :nrb, :], z1, z2)
                    nc.gpsimd.tensor_sub(ds[2][:, :nrb, :], z2, z1)
                    nc.gpsimd.tensor_sub(ds[3][:, :nrb, :], z1, z3)
                dts[bi] = ds

            # --- startup: band0 first, then weights, band1; bias on scalar ring.
            g_sb = wpool.tile([C_IN, CBLKS, KSZ, NPOS, 128], MM_DT)
            bias_sb = wpool.tile([128, CBLKS], mybir.dt.float32)
            stage_band(0)
            nc.sync.dma_start(g_sb[:], g[:])
            stage_band(1)
            nc.scalar.dma_start(bias_sb[:], bias[:, :])

            # HAM pre-warm (see baseline): dummy matmuls on a DVE-zeroed tile.
            warm = wpool.tile([C_IN, 64], MM_DT)
            # share the m0 psum ring so warm fits in the 8 banks
            warm_ps = psum_pool.tile([128, 16, TC], mybir.dt.float32, name="m0")
            nc.vector.memset(warm[:].bitcast(mybir.dt.uint16), 0)
            for _ in range(WARM_COUNT):
                nc.tensor.matmul(
                    warm_ps[:64, 0:2, :], warm[:, :64], warm[:, :56],
                    start=True, stop=True, skip_group_check=True,
                )

            dtrans(0)

            for bi, (b, t) in enumerate(bands):
                if bi + 2 < len(bands):
                    stage_band(bi + 2)
                if bi + 1 < len(bands):
                    dtrans(bi + 1)
                nr = RTS[t]
                ds = dts[bi]
                for cb in range(CBLKS):
                    ms = [
                        psum_pool.tile(
                            [128, 16, TC], mybir.dt.float32, name=f"m{p}"
                        )
                        for p in range(NPOS)
                    ]
                    for pos in range(NPOS):
                        for ky in range(KSZ):
                            a2 = 1 if (t == 0 and ky == 0) else 0
                            b2 = nr - 1 if (t == NT - 1 and ky == 2) else nr
                            nc.tensor.matmul(
                                ms[pos][:, a2:b2, :],
                                g_sb[:, cb, ky, pos, :],
                                ds[pos][:, a2 + ky : b2 + ky, :],
                                start=(ky == 0),
                                stop=(ky == KSZ - 1),
                                skip_group_check=True,
                            )
                    # output transform: y0 = m0+m1+m2+b, y1 = m1-m2-m3+b.
                    # ACT extracts m0/m1+bias/m3 to fp16 SBUF (releases PSUM
                    # early); DVE copies m2 (PSUM copy = 2x, cheaper than
                    # tensor_tensor) then combines at the fp16 SBUF 2x rate.
                    t0 = auxpool.tile([128, 16, TC], mybir.dt.float16)
                    tt = auxpool.tile([128, 16, TC], mybir.dt.float32)
                    t3 = auxpool.tile([128, 16, TC], mybir.dt.float16)
                    uu = auxpool.tile([128, 16, TC], mybir.dt.float16)
                    vv = auxpool.tile([128, 16, TC], mybir.dt.float16)
                    y0 = opool.tile([128, 16, TC], mybir.dt.float16)
                    y1 = opool.tile([128, 16, TC], mybir.dt.float16)
                    idn = mybir.ActivationFunctionType.Identity
                    nc.scalar.activation(t0[:, :nr], ms[0][:, :nr], idn, bias=0.0, scale=1.0)
                    nc.scalar.activation(
                        tt[:, :nr], ms[1][:, :nr], idn, bias=bias_sb[:, cb : cb + 1], scale=1.0
                    )
                    nc.scalar.activation(t3[:, :nr], ms[3][:, :nr], idn, bias=0.0, scale=1.0)
                    nc.vector.tensor_add(uu[:, :nr], ms[2][:, :nr], tt[:, :nr])
                    nc.vector.tensor_sub(vv[:, :nr], tt[:, :nr], ms[2][:, :nr])
                    nc.vector.tensor_add(y0[:, :nr], uu[:, :nr], t0[:, :nr])
                    nc.vector.tensor_sub(y1[:, :nr], vv[:, :nr], t3[:, :nr])
                    r0o = 16 * t
                    nc.sync.dma_start(out[b, cb, 0, :, r0o : r0o + nr, :], y0[:, :nr])
                    nc.sync.dma_start(out[b, cb, 1, :, r0o : r0o + nr, :], y1[:, :nr])
    nc.finalize()
    return nc


def prep_inputs(x, weight, bias):
    # weight (256,128,3,3) -> winograd F(2,3) x-transform, [ci, cb, ky, pos, co_l]
    w6 = weight.reshape(CBLKS, 128, C_IN, KSZ, KSZ).astype(np.float32)
    g0 = w6[..., 0]
    g1 = (w6[..., 0] + w6[..., 1] + w6[..., 2]) * 0.5
    g2 = (w6[..., 0] - w6[..., 1] + w6[..., 2]) * 0.5
    g3 = w6[..., 2]
    G = np.stack([g0, g1, g2, g3], axis=-1)  # [cb, co, ci, ky, pos]
    g_host = np.ascontiguousarray(G.transpose(2, 0, 3, 4, 1), dtype=np.float16)
    bias_r = np.ascontiguousarray(bias.reshape(CBLKS, 128).T, dtype=np.float32)
    xpad = np.zeros((B, C_IN, H, BC), dtype=np.float16)
    xpad[:, :, :, 1:57] = x
    in_maps = []
    for c in range(N_CORES):
        in_maps.append(
            {
                "x": np.ascontiguousarray(xpad[c * B_LOC : (c + 1) * B_LOC]),
                "g": g_host,
                "bias": bias_r,
            }
        )
    return in_maps


_NC_CACHE = {}


def run(x, weight, bias, trace=False, nc=None, tmpdir=None):
    if nc is None:
        nc = _NC_CACHE.get("nc")
        if nc is None:
            nc = _NC_CACHE["nc"] = build_nc()
    in_maps = prep_inputs(np.asarray(x), np.asarray(weight), np.asarray(bias))
    res = run_bass_kernel_spmd(
        nc, in_maps, core_ids=list(range(N_CORES)), trace=trace, tmpdir=tmpdir
    )
    # device out: [B_LOC, CBLKS, 2, 128, 56, 28] fp16 (q = col parity)
    # -> [B, C_OUT, H, W] fp32 on host
    outs = []
    for r in res.results:
        a = np.asarray(r["out"])  # [B_LOC, 2, 2, 128, 56, 28]
        a = a.transpose(0, 1, 3, 4, 5, 2)  # [B_LOC, cb, 128, 56, 28, 2]
        outs.append(a.reshape(B_LOC, C_OUT, H, W))
    out = np.concatenate(outs, axis=0).astype(np.float32)
    return out, res


def kernel(x, weight, bias):
    out, _ = run(x, weight, bias, trace=False)
    return out


if __name__ == "__main__":
    rng = np.random.default_rng(0)
    x = rng.standard_normal((B, C_IN, H, W), dtype=np.float32)
    w = (rng.standard_normal((C_OUT, C_IN, KSZ, KSZ), dtype=np.float32) * 0.05).astype(
        np.float32
    )
    b = rng.standard_normal((C_OUT,), dtype=np.float32)
    out = kernel(x, w, b)
    print(out.shape, out.dtype)


# revision 4
# speedup vs baseline: 1.0462x; 1.0114x over previous
"""Conv2d 3x3 (B=32, Cin=128, H=W=56, Cout=256, pad=1, stride=1) + bias.

1D Winograd F(2,3) along the x-axis, data-parallel over batch across 8 cores
(4 images/core). Per output row, each pair of output columns (2j, 2j+1) is
computed from 4 transformed input planes instead of 6 raw-tap streams:

  z = x[2j-1 .. 2j+2]            (band cols 2j .. 2j+3, col0/col57 zero pad)
  d0 = z0-z2, d1 = z1+z2, d2 = z2-z1, d3 = z1-z3      (input transform, DVE)
  g0 = w0, g1 = (w0+w1+w2)/2, g2 = (w0-w1+w2)/2, g3 = w2   (host)
  m_p[co] = sum_ky sum_ci g_p[ky][ci,co] d_p[ci, r+ky]     (PE, PSUM fp32)
  y0 = m0+m1+m2+bias,  y1 = m1-m2-m3+bias                  (ACT+DVE+GpSimd)

This cuts PE streamed columns per output from 9 to 6 (direct conv streams
each tap separately; winograd shares the m-planes between the two outputs).
The output transform is split across three otherwise-idle engines with at
most one PSUM operand per op (PSUM sources cap DVE at 1x):
  ACT:    t0 = fp16(m0), t = fp32(m1 + bias), t3 = fp16(m3)
  DVE:    u = fp16(m2 + t), [cb0] y0 = u + t0   (+ all 4 input-transform ops)
  GpSimd: v = fp16(t - m2), y1 = v - t3, [cb1] y0 = u + t0

Row tiles of 16 (56 = 16*3 + 8); x staged as overlapping 18-row bands.
Vertical padding handled by clipping matmul rows at the image edges
(PSUM has_written first-touch-overwrite semantics), horizontal padding by
zeroed band columns 0 and 57.
"""

import numpy as np

import concourse.bass as bass
import concourse.mybir as mybir
import concourse.tile as tile
from concourse import bacc
from concourse.bass_utils import run_bass_kernel_spmd

B, C_IN, H, W = 32, 128, 56, 56
C_OUT, KSZ = 256, 3
N_CORES = 8
B_LOC = B // N_CORES  # 4 images per core
RTS = [16, 16, 16, 8]  # output rows per tile (56 = 3*16 + 8)
NT = len(RTS)
CBLKS = C_OUT // 128  # 2
NPOS = 4  # winograd F(2,3) positions
TC = W // 2  # 28 output-column pairs
BC = 58  # band columns (56 + 2 pad)

MM_DT = mybir.dt.float16
WARM_COUNT = 55


def build_nc():
    nc = bacc.Bacc(None, target_bir_lowering=False)
    # x host-padded to 58 cols (zero col 0 and 57) so bands DMA whole rows
    x = nc.dram_tensor("x", [B_LOC, C_IN, H, BC], MM_DT, kind="ExternalInput")
    g = nc.dram_tensor("g", [C_IN, CBLKS, KSZ, NPOS, 128], MM_DT, kind="ExternalInput")
    bias = nc.dram_tensor("bias", [128, CBLKS], mybir.dt.float32, kind="ExternalInput")
    # y0 (even cols) and y1 (odd cols) planes; host interleaves them.
    out = nc.dram_tensor(
        "out", [B_LOC, CBLKS, 2, 128, H, TC], mybir.dt.float16, kind="ExternalOutput"
    )

    bands = [(b, t) for b in range(B_LOC) for t in range(NT)]  # 16 bands

    def band_rows(t):
        r0 = max(0, 16 * t - 1)
        r1 = min(H, 16 * t + RTS[t] + 1)
        l0 = 1 if t == 0 else 0
        return r0, r1, l0

    with tile.TileContext(nc) as tc:
        with (
            tc.tile_pool(name="xin", bufs=5) as xpool,
            tc.tile_pool(name="dpool", bufs=6) as dpool,
            tc.tile_pool(name="wpool", bufs=1) as wpool,
            tc.tile_pool(name="aux", bufs=6) as auxpool,
            tc.tile_pool(name="psum", bufs=2, space="PSUM") as psum_pool,
            tc.tile_pool(name="outp", bufs=8) as opool,
        ):
            xts = {}
            dts = {}

            def stage_band(bi):
                """DMA host-padded x rows for band bi."""
                b, t = bands[bi]
                xt = xpool.tile([C_IN, 18, BC], MM_DT)
                r0, r1, l0 = band_rows(t)
                nc.sync.dma_start(xt[:, l0 : l0 + (r1 - r0), :], x[b, :, r0:r1, :])
                xts[bi] = xt

            def dtrans(bi):
                """Input transform band bi -> 4 d-plane tiles (all on DVE)."""
                _, t = bands[bi]
                nrb = RTS[t] + 2
                xt = xts[bi]
                ds = [
                    dpool.tile([C_IN, 18, TC], MM_DT, name=f"d{p}")
                    for p in range(NPOS)
                ]
                z0 = xt[:, :nrb, 0:55:2]
                z1 = xt[:, :nrb, 1:56:2]
                z2 = xt[:, :nrb, 2:57:2]
                z3 = xt[:, :nrb, 3:58:2]
                # d0 on DVE; d1-d3 on GpSimd (SBUF-only engine, DVE is the
                # only vector engine allowed to touch PSUM so keep its slack).
                # Band 0 is latency-critical: alternate engines so the first
                # group's pos-planes land in matmul order instead of
                # serializing ~3.3us on GpSimd.
                if bi == 0:
                    nc.vector.tensor_sub(ds[0][:, :nrb, :], z0, z2)
                    nc.gpsimd.tensor_add(ds[1][:, :nrb, :], z1, z2)
                    nc.vector.tensor_sub(ds[2][:, :nrb, :], z2, z1)
                    nc.gpsimd.tensor_sub(ds[3][:, :nrb, :], z1, z3)
                else:
                    nc.vector.tensor_sub(ds[0][:, :nrb, :], z0, z2)
                    nc.gpsimd.tensor_add(ds[1][:, :nrb, :], z1, z2)
                    nc.gpsimd.tensor_sub(ds[2][:, :nrb, :], z2, z1)
                    nc.gpsimd.tensor_sub(ds[3][:, :nrb, :], z1, z3)
                dts[bi] = ds

            # --- startup: band0 first, then weights, band1; bias on scalar ring.
            g_sb = wpool.tile([C_IN, CBLKS, KSZ, NPOS, 128], MM_DT)
            bias_sb = wpool.tile([128, CBLKS], mybir.dt.float32)
            stage_band(0)
            nc.sync.dma_start(g_sb[:], g[:])
            stage_band(1)
            nc.scalar.dma_start(bias_sb[:], bias[:, :])

            # HAM pre-warm (see baseline): dummy matmuls on a DVE-zeroed tile.
            warm = wpool.tile([C_IN, 64], MM_DT)
            # share the m0 psum ring so warm fits in the 8 banks
            warm_ps = psum_pool.tile([128, 16, TC], mybir.dt.float32, name="m0")
            nc.vector.memset(warm[:].bitcast(mybir.dt.uint16), 0)
            for _ in range(WARM_COUNT):
                nc.tensor.matmul(
                    warm_ps[:64, 0:2, :], warm[:, :64], warm[:, :56],
                    start=True, stop=True, skip_group_check=True,
                )

            dtrans(0)
            deferred = []

            for bi, (b, t) in enumerate(bands):
                if bi + 2 < len(bands):
                    stage_band(bi + 2)
                if bi + 1 < len(bands):
                    dtrans(bi + 1)
                nr = RTS[t]
                ds = dts[bi]
                for cb in range(CBLKS):
                    ms = [
                        psum_pool.tile(
                            [128, 16, TC], mybir.dt.float32, name=f"m{p}"
                        )
                        for p in range(NPOS)
                    ]
                    for pos in range(NPOS):
                        for ky in range(KSZ):
                            a2 = 1 if (t == 0 and ky == 0) else 0
                            b2 = nr - 1 if (t == NT - 1 and ky == 2) else nr
                            nc.tensor.matmul(
                                ms[pos][:, a2:b2, :],
                                g_sb[:, cb, ky, pos, :],
                                ds[pos][:, a2 + ky : b2 + ky, :],
                                start=(ky == 0),
                                stop=(ky == KSZ - 1),
                                skip_group_check=True,
                            )
                    # output transform: y0 = m0+m1+m2+b, y1 = m1-m2-m3+b.
                    # ACT extracts m0/m1+bias/m3 to fp16 SBUF (releases PSUM
                    # early); DVE copies m2 (PSUM copy = 2x, cheaper than
                    # tensor_tensor) then combines at the fp16 SBUF 2x rate.
                    t0 = auxpool.tile([128, 16, TC], mybir.dt.float16)
                    tt = auxpool.tile([128, 16, TC], mybir.dt.float32)
                    t3 = auxpool.tile([128, 16, TC], mybir.dt.float16)
                    uu = auxpool.tile([128, 16, TC], mybir.dt.float16)
                    vv = auxpool.tile([128, 16, TC], mybir.dt.float16)
                    y0 = opool.tile([128, 16, TC], mybir.dt.float16)
                    y1 = opool.tile([128, 16, TC], mybir.dt.float16)
                    idn = mybir.ActivationFunctionType.Identity
                    nc.scalar.activation(t0[:, :nr], ms[0][:, :nr], idn, bias=0.0, scale=1.0)
                    nc.scalar.activation(
                        tt[:, :nr], ms[1][:, :nr], idn, bias=bias_sb[:, cb : cb + 1], scale=1.0
                    )
                    nc.scalar.activation(t3[:, :nr], ms[3][:, :nr], idn, bias=0.0, scale=1.0)
                    nc.vector.tensor_add(uu[:, :nr], ms[2][:, :nr], tt[:, :nr])
                    nc.vector.tensor_sub(vv[:, :nr], tt[:, :nr], ms[2][:, :nr])
                    nc.vector.tensor_add(y0[:, :nr], uu[:, :nr], t0[:, :nr])
                    nc.vector.tensor_sub(y1[:, :nr], vv[:, :nr], t3[:, :nr])
                    r0o = 16 * t
                    if bi == len(bands) - 1:
                        # tail: defer y0 triggers to scalar AFTER all ACT work
                        deferred.append((b, cb, r0o, nr, y0))
                        nc.sync.dma_start(
                            out[b, cb, 1, :, r0o : r0o + nr, :], y1[:, :nr]
                        )
                    else:
                        nc.sync.dma_start(
                            out[b, cb, 0, :, r0o : r0o + nr, :], y0[:, :nr]
                        )
                        nc.sync.dma_start(
                            out[b, cb, 1, :, r0o : r0o + nr, :], y1[:, :nr]
                        )
            for b_d, cb_d, r0o_d, nr_d, y0_d in deferred:
                nc.scalar.dma_start(
                    out[b_d, cb_d, 0, :, r0o_d : r0o_d + nr_d, :], y0_d[:, :nr_d]
                )
    nc.finalize()
    return nc


def prep_inputs(x, weight, bias):
    # weight (256,128,3,3) -> winograd F(2,3) x-transform, [ci, cb, ky, pos, co_l]
    w6 = weight.reshape(CBLKS, 128, C_IN, KSZ, KSZ).astype(np.float32)
    g0 = w6[..., 0]
    g1 = (w6[..., 0] + w6[..., 1] + w6[..., 2]) * 0.5
    g2 = (w6[..., 0] - w6[..., 1] + w6[..., 2]) * 0.5
    g3 = w6[..., 2]
    G = np.stack([g0, g1, g2, g3], axis=-1)  # [cb, co, ci, ky, pos]
    g_host = np.ascontiguousarray(G.transpose(2, 0, 3, 4, 1), dtype=np.float16)
    bias_r = np.ascontiguousarray(bias.reshape(CBLKS, 128).T, dtype=np.float32)
    xpad = np.zeros((B, C_IN, H, BC), dtype=np.float16)
    xpad[:, :, :, 1:57] = x
    in_maps = []
    for c in range(N_CORES):
        in_maps.append(
            {
                "x": np.ascontiguousarray(xpad[c * B_LOC : (c + 1) * B_LOC]),
                "g": g_host,
                "bias": bias_r,
            }
        )
    return in_maps


_NC_CACHE = {}


def run(x, weight, bias, trace=False, nc=None, tmpdir=None):
    if nc is None:
        nc = _NC_CACHE.get("nc")
        if nc is None:
            nc = _NC_CACHE["nc"] = build_nc()
    in_maps = prep_inputs(np.asarray(x), np.asarray(weight), np.asarray(bias))
    res = run_bass_kernel_spmd(
        nc, in_maps, core_ids=list(range(N_CORES)), trace=trace, tmpdir=tmpdir
    )
    # device out: [B_LOC, CBLKS, 2, 128, 56, 28] fp16 (q = col parity)
    # -> [B, C_OUT, H, W] fp32 on host
    outs = []
    for r in res.results:
        a = np.asarray(r["out"])  # [B_LOC, 2, 2, 128, 56, 28]
        a = a.transpose(0, 1, 3, 4, 5, 2)  # [B_LOC, cb, 128, 56, 28, 2]
        outs.append(a.reshape(B_LOC, C_OUT, H, W))
    out = np.concatenate(outs, axis=0).astype(np.float32)
    return out, res


def kernel(x, weight, bias):
    out, _ = run(x, weight, bias, trace=False)
    return out


if __name__ == "__main__":
    rng = np.random.default_rng(0)
    x = rng.standard_normal((B, C_IN, H, W), dtype=np.float32)
    w = (rng.standard_normal((C_OUT, C_IN, KSZ, KSZ), dtype=np.float32) * 0.05).astype(
        np.float32
    )
    b = rng.standard_normal((C_OUT,), dtype=np.float32)
    out = kernel(x, w, b)
    print(out.shape, out.dtype)


# revision 5
# speedup vs baseline: 1.0674x; 1.0202x over previous
"""Conv2d 3x3 (B=32, Cin=128, H=W=56, Cout=256, pad=1, stride=1) + bias.

1D Winograd F(2,3) along the x-axis, data-parallel over batch across 8 cores
(4 images/core). Per output row, each pair of output columns (2j, 2j+1) is
computed from 4 transformed input planes instead of 6 raw-tap streams:

  z = x[2j-1 .. 2j+2]            (band cols 2j .. 2j+3, col0/col57 zero pad)
  d0 = z0-z2, d1 = z1+z2, d2 = z2-z1, d3 = z1-z3      (input transform, DVE)
  g0 = w0, g1 = (w0+w1+w2)/2, g2 = (w0-w1+w2)/2, g3 = w2   (host)
  m_p[co] = sum_ky sum_ci g_p[ky][ci,co] d_p[ci, r+ky]     (PE, PSUM fp32)
  y0 = m0+m1+m2+bias,  y1 = m1-m2-m3+bias                  (ACT+DVE+GpSimd)

This cuts PE streamed columns per output from 9 to 6 (direct conv streams
each tap separately; winograd shares the m-planes between the two outputs).
The output transform is split across three otherwise-idle engines with at
most one PSUM operand per op (PSUM sources cap DVE at 1x):
  ACT:    t0 = fp16(m0), t = fp32(m1 + bias), t3 = fp16(m3)
  DVE:    u = fp16(m2 + t), [cb0] y0 = u + t0   (+ all 4 input-transform ops)
  GpSimd: v = fp16(t - m2), y1 = v - t3, [cb1] y0 = u + t0

Row tiles of 16 (56 = 16*3 + 8); x staged as overlapping 18-row bands.
Vertical padding handled by clipping matmul rows at the image edges
(PSUM has_written first-touch-overwrite semantics), horizontal padding by
zeroed band columns 0 and 57.
"""

import numpy as np

import concourse.bass as bass
import concourse.mybir as mybir
import concourse.tile as tile
from concourse import bacc
from concourse.bass_utils import run_bass_kernel_spmd

B, C_IN, H, W = 32, 128, 56, 56
C_OUT, KSZ = 256, 3
N_CORES = 8
B_LOC = B // N_CORES  # 4 images per core
RTS = [16, 16, 16, 8]  # output rows per tile (56 = 3*16 + 8)
NT = len(RTS)
CBLKS = C_OUT // 128  # 2
NPOS = 4  # winograd F(2,3) positions
TC = W // 2  # 28 output-column pairs
BC = 58  # band columns (56 + 2 pad)

MM_DT = mybir.dt.float16
WARM_COUNT = 55


def build_nc():
    nc = bacc.Bacc(None, target_bir_lowering=False)
    # x host-padded to 58 cols (zero col 0 and 57) so bands DMA whole rows
    x = nc.dram_tensor("x", [B_LOC, C_IN, H, BC], MM_DT, kind="ExternalInput")
    g = nc.dram_tensor("g", [C_IN, CBLKS, KSZ, NPOS, 128], MM_DT, kind="ExternalInput")
    bias = nc.dram_tensor("bias", [128, CBLKS], mybir.dt.float32, kind="ExternalInput")
    # y0 (even cols) and y1 (odd cols) planes; host interleaves them.
    out = nc.dram_tensor(
        "out", [B_LOC, CBLKS, 2, 128, H, TC], mybir.dt.float16, kind="ExternalOutput"
    )

    bands = [(b, t) for b in range(B_LOC) for t in range(NT)]  # 16 bands

    def band_rows(t):
        r0 = max(0, 16 * t - 1)
        r1 = min(H, 16 * t + RTS[t] + 1)
        l0 = 1 if t == 0 else 0
        return r0, r1, l0

    with tile.TileContext(nc) as tc:
        with (
            tc.tile_pool(name="xin", bufs=5) as xpool,
            tc.tile_pool(name="dpool", bufs=6) as dpool,
            tc.tile_pool(name="wpool", bufs=1) as wpool,
            tc.tile_pool(name="aux", bufs=4) as auxpool,
            tc.tile_pool(name="psum", bufs=2, space="PSUM") as psum_pool,
            tc.tile_pool(name="outp", bufs=8) as opool,
        ):
            xts = {}
            dts = {}

            def stage_band(bi):
                """DMA host-padded x rows for band bi."""
                b, t = bands[bi]
                xt = xpool.tile([C_IN, 18, BC], MM_DT)
                r0, r1, l0 = band_rows(t)
                nc.sync.dma_start(xt[:, l0 : l0 + (r1 - r0), :], x[b, :, r0:r1, :])
                xts[bi] = xt

            def dtrans(bi):
                """Input transform band bi -> 4 d-plane tiles (all on DVE)."""
                _, t = bands[bi]
                nrb = RTS[t] + 2
                xt = xts[bi]
                ds = [
                    dpool.tile([C_IN, 18, TC], MM_DT, name=f"d{p}")
                    for p in range(NPOS)
                ]
                z0 = xt[:, :nrb, 0:55:2]
                z1 = xt[:, :nrb, 1:56:2]
                z2 = xt[:, :nrb, 2:57:2]
                z3 = xt[:, :nrb, 3:58:2]
                # d0 on DVE; d1-d3 on GpSimd (SBUF-only engine, DVE is the
                # only vector engine allowed to touch PSUM so keep its slack).
                # Band 0 is latency-critical: alternate engines so the first
                # group's pos-planes land in matmul order instead of
                # serializing ~3.3us on GpSimd.
                if bi == 0:
                    nc.vector.tensor_sub(ds[0][:, :nrb, :], z0, z2)
                    nc.gpsimd.tensor_add(ds[1][:, :nrb, :], z1, z2)
                    nc.vector.tensor_sub(ds[2][:, :nrb, :], z2, z1)
                    nc.gpsimd.tensor_sub(ds[3][:, :nrb, :], z1, z3)
                else:
                    nc.vector.tensor_sub(ds[0][:, :nrb, :], z0, z2)
                    nc.gpsimd.tensor_add(ds[1][:, :nrb, :], z1, z2)
                    nc.gpsimd.tensor_sub(ds[2][:, :nrb, :], z2, z1)
                    nc.gpsimd.tensor_sub(ds[3][:, :nrb, :], z1, z3)
                dts[bi] = ds

            # --- startup: band0 first, then weights, band1; bias on scalar ring.
            g_sb = wpool.tile([C_IN, CBLKS, KSZ, NPOS, 128], MM_DT)
            bias_sb = wpool.tile([128, CBLKS], mybir.dt.float32)
            stage_band(0)
            nc.sync.dma_start(g_sb[:], g[:])
            stage_band(1)
            nc.scalar.dma_start(bias_sb[:], bias[:, :])

            # HAM pre-warm (see baseline): dummy matmuls on a DVE-zeroed tile.
            warm = wpool.tile([C_IN, 64], MM_DT)
            # share the m0 psum ring so warm fits in the 8 banks
            warm_ps = psum_pool.tile([128, 16, TC], mybir.dt.float32, name="m0")
            nc.vector.memset(warm[:].bitcast(mybir.dt.uint16), 0)
            for _ in range(WARM_COUNT):
                nc.tensor.matmul(
                    warm_ps[:64, 0:2, :], warm[:, :64], warm[:, :56],
                    start=True, stop=True, skip_group_check=True,
                )

            dtrans(0)
            deferred = []

            for bi, (b, t) in enumerate(bands):
                if bi + 2 < len(bands):
                    stage_band(bi + 2)
                if bi + 1 < len(bands):
                    dtrans(bi + 1)
                nr = RTS[t]
                ds = dts[bi]
                t0d = auxpool.tile([128, 2, 16, TC], mybir.dt.float16)
                ttd = auxpool.tile([128, 2, 16, TC], mybir.dt.float32)
                t3d = auxpool.tile([128, 2, 16, TC], mybir.dt.float16)
                uud = auxpool.tile([128, 2, 16, TC], mybir.dt.float16)
                vvd = auxpool.tile([128, 2, 16, TC], mybir.dt.float16)
                y0d = opool.tile([128, 2, 16, TC], mybir.dt.float16)
                y1d = opool.tile([128, 2, 16, TC], mybir.dt.float16)
                for cb in range(CBLKS):
                    ms = [
                        psum_pool.tile(
                            [128, 16, TC], mybir.dt.float32, name=f"m{p}"
                        )
                        for p in range(NPOS)
                    ]
                    for pos in range(NPOS):
                        for ky in range(KSZ):
                            a2 = 1 if (t == 0 and ky == 0) else 0
                            b2 = nr - 1 if (t == NT - 1 and ky == 2) else nr
                            nc.tensor.matmul(
                                ms[pos][:, a2:b2, :],
                                g_sb[:, cb, ky, pos, :],
                                ds[pos][:, a2 + ky : b2 + ky, :],
                                start=(ky == 0),
                                stop=(ky == KSZ - 1),
                                skip_group_check=True,
                            )
                    # output transform: y0 = m0+m1+m2+b, y1 = m1-m2-m3+b.
                    # per-cb PSUM extracts into halves of double-width tiles;
                    # the final combines run once per band at 2x FD.
                    idn = mybir.ActivationFunctionType.Identity
                    nc.scalar.activation(
                        t0d[:, cb, :nr], ms[0][:, :nr], idn, bias=0.0, scale=1.0
                    )
                    nc.scalar.activation(
                        ttd[:, cb, :nr], ms[1][:, :nr], idn,
                        bias=bias_sb[:, cb : cb + 1], scale=1.0,
                    )
                    nc.scalar.activation(
                        t3d[:, cb, :nr], ms[3][:, :nr], idn, bias=0.0, scale=1.0
                    )
                    nc.vector.tensor_add(uud[:, cb, :nr], ms[2][:, :nr], ttd[:, cb, :nr])
                    nc.vector.tensor_sub(vvd[:, cb, :nr], ttd[:, cb, :nr], ms[2][:, :nr])
                # band-wide combines (both cb halves in one op)
                nc.vector.tensor_add(
                    y0d[:, :, :nr, :], uud[:, :, :nr, :], t0d[:, :, :nr, :]
                )
                nc.vector.tensor_sub(
                    y1d[:, :, :nr, :], vvd[:, :, :nr, :], t3d[:, :, :nr, :]
                )
                r0o = 16 * t
                for cb in range(CBLKS):
                    if bi == len(bands) - 1:
                        # tail: defer y0 triggers to scalar AFTER all ACT work
                        deferred.append((b, cb, r0o, nr, y0d))
                        nc.sync.dma_start(
                            out[b, cb, 1, :, r0o : r0o + nr, :], y1d[:, cb, :nr]
                        )
                    else:
                        nc.sync.dma_start(
                            out[b, cb, 0, :, r0o : r0o + nr, :], y0d[:, cb, :nr]
                        )
                        nc.sync.dma_start(
                            out[b, cb, 1, :, r0o : r0o + nr, :], y1d[:, cb, :nr]
                        )
            for b_d, cb_d, r0o_d, nr_d, y0_d in deferred:
                nc.scalar.dma_start(
                    out[b_d, cb_d, 0, :, r0o_d : r0o_d + nr_d, :],
                    y0_d[:, cb_d, :nr_d],
                )
    nc.finalize()
    return nc


def prep_inputs(x, weight, bias):
    # weight (256,128,3,3) -> winograd F(2,3) x-transform, [ci, cb, ky, pos, co_l]
    w6 = weight.reshape(CBLKS, 128, C_IN, KSZ, KSZ).astype(np.float32)
    g0 = w6[..., 0]
    g1 = (w6[..., 0] + w6[..., 1] + w6[..., 2]) * 0.5
    g2 = (w6[..., 0] - w6[..., 1] + w6[..., 2]) * 0.5
    g3 = w6[..., 2]
    G = np.stack([g0, g1, g2, g3], axis=-1)  # [cb, co, ci, ky, pos]
    g_host = np.ascontiguousarray(G.transpose(2, 0, 3, 4, 1), dtype=np.float16)
    bias_r = np.ascontiguousarray(bias.reshape(CBLKS, 128).T, dtype=np.float32)
    xpad = np.zeros((B, C_IN, H, BC), dtype=np.float16)
    xpad[:, :, :, 1:57] = x
    in_maps = []
    for c in range(N_CORES):
        in_maps.append(
            {
                "x": np.ascontiguousarray(xpad[c * B_LOC : (c + 1) * B_LOC]),
                "g": g_host,
                "bias": bias_r,
            }
        )
    return in_maps


_NC_CACHE = {}


def run(x, weight, bias, trace=False, nc=None, tmpdir=None):
    if nc is None:
        nc = _NC_CACHE.get("nc")
        if nc is None:
            nc = _NC_CACHE["nc"] = build_nc()
    in_maps = prep_inputs(np.asarray(x), np.asarray(weight), np.asarray(bias))
    res = run_bass_kernel_spmd(
        nc, in_maps, core_ids=list(range(N_CORES)), trace=trace, tmpdir=tmpdir
    )
    # device out: [B_LOC, CBLKS, 2, 128, 56, 28] fp16 (q = col parity)
    # -> [B, C_OUT, H, W] fp32 on host
    outs = []
    for r in res.results:
        a = np.asarray(r["out"])  # [B_LOC, 2, 2, 128, 56, 28]
        a = a.transpose(0, 1, 3, 4, 5, 2)  # [B_LOC, cb, 128, 56, 28, 2]
        outs.append(a.reshape(B_LOC, C_OUT, H, W))
    out = np.concatenate(outs, axis=0).astype(np.float32)
    return out, res


def kernel(x, weight, bias):
    out, _ = run(x, weight, bias, trace=False)
    return out


if __name__ == "__main__":
    rng = np.random.default_rng(0)
    x = rng.standard_normal((B, C_IN, H, W), dtype=np.float32)
    w = (rng.standard_normal((C_OUT, C_IN, KSZ, KSZ), dtype=np.float32) * 0.05).astype(
        np.float32
    )
    b = rng.standard_normal((C_OUT,), dtype=np.float32)
    out = kernel(x, w, b)
    print(out.shape, out.dtype)
